# revision 1
# baseline (speedup 1.0000x reference)
"""Causal GQA attention block (RoPE, 32 q-heads / 8 kv-heads, fp32 I/O) on
8 Trainium2 NeuronCores — v2, DMA-count-optimized.

Sharding: sequence-parallel (as v1): core c owns batch b=c//4 and query
blocks {j, 7-j}, j=c%4 (256 tokens each; causal work balances). k/v are
all-gathered within each 4-core batch group in TWO collectives (kv heads
0-3, then 4-7) so the first gather overlaps the q projection.

Key layout trick: kv head kv lives at partition rows (kv%2)*64 for BOTH
its k features and its 4 q heads, so every score matmul has lhsT and rhs
at the same base partition — no k row duplication anywhere.

v ships with a ones-column per head (65-wide blocks) baked in at
projection-drain time, so the AV matmul's 65th output row accumulates
the softmax denominator for free.

DMA discipline: every HWDGE dma_start costs ~625ns on a globally
serialized descriptor engine, so transfers are slabs: 16 x-chunks,
16+4+16 qkv weight col-slabs, 16 Wo row-slabs (SBUF-resident, loaded
during attention), 16 k + 32 v gathered-tile loads, 16 output stores.
"""

import sys
import json

sys.path.insert(0, "/opt/trn_rl_repo")

import numpy as np
import ml_dtypes

import concourse.bass as bass
import concourse.tile as tile
from concourse import mybir

F32 = mybir.dt.float32
BF16 = mybir.dt.bfloat16
BF = ml_dtypes.bfloat16
AF = mybir.ActivationFunctionType

# ---------------------------------------------------------------------------
# walrus workaround: this build supports one semaphore wait per instruction,
# but TileContext's tail drain attaches several. Split the extras onto
# standalone EventSemaphore instructions placed just before the instruction.
# ---------------------------------------------------------------------------


def _fix_multiwait(bir_bytes):
    d = json.loads(bir_bytes)
    ctr = 0
    changed = False
    for fn in d.get("functions", []):
        for blk in fn.get("blocks", []):
            new_insts = []
            for inst in blk["instructions"]:
                si = inst.get("sync_info") or {}
                waits = si.get("on_wait") or []
                if len(waits) > 1:
                    changed = True
                    for w in waits[:-1]:
                        ctr += 1
                        new_insts.append({
                            "debug": inst.get("debug", 0),
                            "engine": inst["engine"],
                            "ins": [],
                            "name": f"mwfix_{ctr}_{inst['name']}",
                            "opcode": "EventSemaphore",
                            "outs": [],
                            "sync_info": {"on_update": [], "on_wait": [w]},
                        })
                    si["on_wait"] = [waits[-1]]
                new_insts.append(inst)
            blk["instructions"] = new_insts
    return json.dumps(d).encode() if changed else bir_bytes


def _install_birfix():
    from concourse import bass_utils, bass2jax

    if getattr(bass_utils, "_mwfix_installed", False):
        return
    orig = bass_utils.compile_bir_kernel

    def patched(bir_json, tmpdir, neff_name="file.neff", **kw):
        if isinstance(bir_json, str):
            bir_json = bir_json.encode()
        return orig(_fix_multiwait(bir_json), tmpdir, neff_name, **kw)

    bass_utils.compile_bir_kernel = patched
    bass_utils._mwfix_installed = True
    bass2jax.compile_bir_kernel = patched


# ---------------------------------------------------------------------------
# configuration
# ---------------------------------------------------------------------------


class Cfg:
    def __init__(self, B=2, T=2048, DIM=2048, NH=32, NKV=8, HD=64,
                 rope_base=10000.0):
        self.B, self.T, self.DIM = B, T, DIM
        self.NH, self.NKV, self.HD = NH, NKV, HD
        self.rope_base = rope_base
        self.NCORES = 8
        self.BLK = T // 8            # tokens per query block (256)
        self.KT = self.BLK // 2      # tokens per k-tile (128)
        self.TOK = 2 * self.BLK      # tokens per core (512)
        self.KDIM = NKV * HD         # 512
        self.GQ = NH // NKV          # q heads per kv head (4)
        self.NKT = 16                # k-tiles in a full sequence
        self.NCT = DIM // 128        # contraction chunks (16)
        self.HD2 = HD // 2           # 32
        # bounce sizes (per quarter = 2 kv heads)
        self.KQTR = 2 * HD * self.TOK           # 128x512 feature-major
        self.VQTR = self.TOK * 2 * (HD + 1)     # 512x130 token-major
        self.QTR = self.KQTR + self.VQTR


FULL = Cfg()


def core_blocks(c):
    return c // 4, c % 4, 7 - (c % 4)


def ktile_src(cfg, g):
    """k-tile g (tokens [g*KT,(g+1)*KT)) -> (owner group-slot, col base)."""
    i = g // 2
    jj = min(i, 7 - i)
    colbase = (0 if i == jj else cfg.BLK) + (g % 2) * cfg.KT
    return jj, colbase


# ---------------------------------------------------------------------------
# device program
# ---------------------------------------------------------------------------


def build_nc(cfg: Cfg, reps=1):
    c = cfg
    nc = bass.Bass(num_devices=c.NCORES)

    # weights ship host-pre-tiled so every slab DMA reads contiguous
    # multi-KB runs per partition row (128 descriptors, not 2048):
    #   xT   [128, ct*TOK + t]        = x^T[ct*128+r, t]
    #   wq_t [128, p*2048 + ct*128+o] = Wq^T[ct*128+r, p*128+o]   (rope-perm)
    #   wk_t [128, p*2048 + ct*128+o] = Wk^T[ct*128+r, p*128+o]   (rope-perm)
    #   wv_t [128, ct*KDIM + o]       = Wv^T[ct*128+r, o]
    #   wo_t [128, ct*DIM + o]        = Wo^T[ct*128+r, o]
    xT = nc.declare_dram_parameter("xT", [128, c.NCT * c.TOK], BF16,
                                   isOutput=False)
    wqT = nc.declare_dram_parameter("wqT", [128, (c.NH // 2) * c.NCT * 128],
                                    BF16, isOutput=False)
    wkT = nc.declare_dram_parameter("wkT", [128, 4 * c.NCT * 128], BF16,
                                    isOutput=False)
    wvT = nc.declare_dram_parameter("wvT", [128, c.NCT * c.KDIM], BF16,
                                    isOutput=False)
    woT = nc.declare_dram_parameter("woT", [128, c.NCT * c.DIM], BF16,
                                    isOutput=False)
    # rope tables duplicated across both 64-row bands so every DVE mul has
    # both SBUF inputs at the same start partition (walrus requires it):
    # csA rows = [cos, sin, cos, sin] x 32, csB rows = [sin, cos, sin, cos].
    csA = nc.declare_dram_parameter("csA", [128, c.TOK], BF16, isOutput=False)
    csB = nc.declare_dram_parameter("csB", [128, c.TOK], BF16, isOutput=False)
    # masks[kk, g*256:(g+1)*256] — per k-tile g, the (A if g<8 else B)-block
    # causal mask; broadcast across the head pair at use site (stride-0).
    masks = nc.declare_dram_parameter("masks", [c.KT, c.NKT * c.BLK], BF16,
                                      isOutput=False)
    out = nc.declare_dram_parameter("out", [c.TOK, c.DIM], BF16, isOutput=True)

    bounce = [nc.dram_tensor(f"bounce{q}", [c.QTR], BF16) for q in range(4)]
    allg = [nc.dram_tensor(f"allg{q}", [4, c.QTR], BF16) for q in range(4)]

    def k_qtr_view(ap):    # [128 feat, TOK] feature-major
        return ap[0:c.KQTR].rearrange("(f t) -> f t", t=c.TOK)

    def v_qtr_view(ap):    # [TOK, 130] token-major
        return ap[c.KQTR:c.QTR].rearrange("(t f) -> t f", f=2 * (c.HD + 1))

    V65 = c.HD + 1

    with tile.TileContext(nc) as tc:
        with tc.tile_pool(name="glob", bufs=1) as glob:
            # persistent across phases
            qT_sb = glob.tile([128, c.NKV * c.GQ // 2 * c.TOK], BF16)  # [128, 8192]
            yT_sb = glob.tile([128, c.NCT * c.TOK], BF16)
            # per-quarter tiles so kv (2q, 2q+1) only depends on gather q
            k_sb = [glob.tile([128, 4 * c.TOK], BF16, name=f"k_sb_{q}")
                    for q in range(4)]
            v65_sb = [glob.tile([128, 16 * 2 * V65], BF16, name=f"v65_{q}")
                      for q in range(4)]
            csA_sb = glob.tile([128, c.TOK], BF16)
            csB_sb = glob.tile([128, c.TOK], BF16)
            mask_sb = glob.tile([c.KT, c.NKT * c.BLK], BF16)
            nc.sync.dma_start(csA_sb[:], csA[:])
            nc.sync.dma_start(csB_sb[:], csB[:])
            nc.sync.dma_start(mask_sb[:], masks[:])

            def q_ap(kv, hq, r0=None):
                """q head (kv, hq∈0..3): [64, TOK] at rows (kv%2)*64.
                kv pairs (2i, 2i+1) share column block i (row bands 0/64)."""
                r = (kv % 2) * 64 if r0 is None else r0
                col = ((kv // 2) * c.GQ + hq) * c.TOK
                return qT_sb[r:r + 64, col:col + c.TOK]

            def yhead_ap(h):
                a, r = h // 2, (h % 2) * c.HD
                return yT_sb[r:r + c.HD, a * c.TOK:(a + 1) * c.TOK]

            for _rep in range(reps):
                # ================= phase 1: projections + rope =============
                with tc.tile_pool(name="proj", bufs=1) as proj, \
                     tc.tile_pool(name="wstr", bufs=4) as wstr, \
                     tc.tile_pool(name="ppool", bufs=3, space="PSUM") as ppool, \
                     tc.tile_pool(name="vps", bufs=4, space="PSUM") as vps, \
                     tc.tile_pool(name="dr", bufs=8) as dr, \
                     tc.tile_pool(name="rt", bufs=2) as rt:

                    xT_sb = proj.tile([128, c.NCT * c.TOK], BF16)
                    kT_sb = proj.tile([128, 4 * c.TOK], BF16)
                    wq_res = proj.tile([128, (c.NH // 2) * c.NCT * 128], BF16)
                    v65loc = [proj.tile([128, 4 * 2 * V65], BF16,
                                        name=f"v65loc_{q}_{_rep}")
                              for q in range(4)]
                    # wk0 first, then x in 4 streamed pieces: the first
                    # k-proj matmul can start ~3us in instead of ~12us.
                    wk_sbs = []
                    for ot in range(4):
                        wk_sb = wstr.tile([128, c.NCT * 128], BF16, tag="wk",
                                          name=f"wk_sb_{_rep}_{ot}")
                        wk_sbs.append(wk_sb)
                    nc.sync.dma_start(
                        wk_sbs[0][:], wkT[:, 0:c.NCT * 128])
                    QX = c.NCT // 4
                    for piece in range(4):
                        nc.sync.dma_start(
                            xT_sb[:, piece * QX * c.TOK:(piece + 1) * QX * c.TOK],
                            xT[:, piece * QX * c.TOK:(piece + 1) * QX * c.TOK])
                    for ot in range(1, 4):
                        nc.sync.dma_start(
                            wk_sbs[ot][:],
                            wkT[:, ot * c.NCT * 128:(ot + 1) * c.NCT * 128])

                    def xt_chunk(ct):
                        return xT_sb[:, ct * c.TOK:(ct + 1) * c.TOK]

                    def rope_pair(ps, dst_of_hh):
                        """psum tile [128, TOK] = 2 heads x [ev32; od32].
                        Drain to bf16 on ACT, rotate on DVE into dst. The
                        dr pool is deep so drains run at matmul pace and
                        clear the in-order Act queue before attention."""
                        d = dr.tile([128, c.TOK], BF16, tag="d")
                        nc.scalar.copy(d[:], ps[:])
                        for hh in range(2):
                            ev = d[hh * 64:hh * 64 + 32, :]
                            od = d[hh * 64 + 32:hh * 64 + 64, :]
                            cos_e = csA_sb[hh * 64:hh * 64 + 32, :]
                            sin_o = csA_sb[hh * 64 + 32:hh * 64 + 64, :]
                            sin_e = csB_sb[hh * 64:hh * 64 + 32, :]
                            cos_o = csB_sb[hh * 64 + 32:hh * 64 + 64, :]
                            t1 = rt.tile([c.HD2, c.TOK], BF16, tag="t1")
                            t2 = rt.tile([c.HD2, c.TOK], BF16, tag="t2")
                            nc.vector.tensor_mul(t1[:], ev, cos_e)
                            nc.vector.tensor_mul(t2[:], od, sin_o)
                            nc.vector.tensor_sub(dst_of_hh(hh, 0), t1[:], t2[:])
                            t3 = rt.tile([c.HD2, c.TOK], BF16, tag="t1",
                                         name="t3")
                            t4 = rt.tile([c.HD2, c.TOK], BF16, tag="t2",
                                         name="t4")
                            nc.vector.tensor_mul(t3[:], ev, sin_e)
                            nc.vector.tensor_mul(t4[:], od, cos_o)
                            nc.vector.tensor_add(dst_of_hh(hh, 32), t3[:], t4[:])

                    def kproj(ot):
                        ps = ppool.tile([128, c.TOK], F32, tag="p")
                        for ct in range(c.NCT):
                            nc.tensor.matmul(
                                ps[:], wk_sbs[ot][:, ct * 128:(ct + 1) * 128],
                                xt_chunk(ct),
                                start=(ct == 0), stop=(ct == c.NCT - 1))

                        def kdst(hh, r0, ot=ot):
                            kv = 2 * ot + hh
                            rr = (kv % 2) * 64 + r0
                            return kT_sb[rr:rr + 32,
                                         ot * c.TOK:(ot + 1) * c.TOK]
                        rope_pair(ps, kdst)

                    def vdrain(q):
                        # psum -> bf16 on DVE (Act is busy with rope drains)
                        for tt in range(c.TOK // 128):
                            for q2 in range(2):
                                kv = q * 2 + q2
                                nc.vector.tensor_copy(
                                    v65loc[q][:, tt * 2 * V65 + q2 * V65:
                                              tt * 2 * V65 + q2 * V65 + c.HD],
                                    psvs[tt][:, kv * c.HD:(kv + 1) * c.HD])

                    def gather(q):
                        nc.sync.dma_start(
                            k_qtr_view(bounce[q]),
                            kT_sb[:, q * c.TOK:(q + 1) * c.TOK])
                        for tt in range(c.TOK // 128):
                            nc.sync.dma_start(
                                v_qtr_view(bounce[q])[
                                    tt * 128:(tt + 1) * 128, :],
                                v65loc[q][:, tt * 2 * V65:(tt + 1) * 2 * V65])
                        nc.gpsimd.collective_compute(
                            "AllGather", mybir.AluOpType.bypass,
                            replica_groups=[[0, 1, 2, 3], [4, 5, 6, 7]],
                            ins=[bounce[q][:]], outs=[allg[q][:]])

                    # ---- k pairs 0,1, then v (all), then quarter gathers
                    kproj(0)
                    kproj(1)
                    vw_sb = proj.tile([128, c.NCT * c.KDIM], BF16)
                    nc.sync.dma_start(vw_sb[:], wvT[:])
                    psvs = [vps.tile([128, c.KDIM], F32, tag="v",
                                     name=f"psv_{_rep}_{i}")
                            for i in range(c.TOK // 128)]
                    for ct in range(c.NCT):
                        for tt in range(c.TOK // 128):
                            nc.tensor.matmul(
                                psvs[tt][:],
                                xt_chunk(ct)[:, tt * 128:(tt + 1) * 128],
                                vw_sb[:, ct * c.KDIM:(ct + 1) * c.KDIM],
                                start=(ct == 0), stop=(ct == c.NCT - 1))
                    for q in range(4):
                        nc.vector.memset(v65loc[q][:], 1.0)
                    vdrain(0)
                    gather(0)
                    vdrain(1)
                    gather(1)
                    kproj(2)
                    vdrain(2)
                    gather(2)
                    kproj(3)
                    vdrain(3)
                    gather(3)

                    # ---- q projection + rope (overlaps the gathers);
                    # wq resident, streamed in per-pair slabs so the big
                    # transfers don't starve the bounce DMAs ----
                    for p in range(c.NH // 2):
                        nc.sync.dma_start(
                            wq_res[:, p * c.NCT * 128:(p + 1) * c.NCT * 128],
                            wqT[:, p * c.NCT * 128:(p + 1) * c.NCT * 128])
                        ps = ppool.tile([128, c.TOK], F32, tag="p")
                        for ct in range(c.NCT):
                            nc.tensor.matmul(
                                ps[:],
                                wq_res[:, p * c.NCT * 128 + ct * 128:
                                       p * c.NCT * 128 + (ct + 1) * 128],
                                xt_chunk(ct),
                                start=(ct == 0), stop=(ct == c.NCT - 1))

                        # slab p = (i, hq) holds head 8i+hq (psum rows 0:64,
                        # kv even) and head 8i+4+hq (rows 64:128, kv odd) —
                        # both land in qT column block p with no partition
                        # shift (host orders Wq columns accordingly).
                        def qdst(hh, r0, p=p):
                            rr = hh * 64 + r0
                            col = p * c.TOK
                            return qT_sb[rr:rr + 32, col:col + c.TOK]
                        rope_pair(ps, qdst)

                # ================= phase 2: attention ======================
                with tc.tile_pool(name="wo", bufs=1) as wo_pool:
                    wo_res = wo_pool.tile([128, c.NCT * c.DIM], BF16)
                    for ct in range(c.NCT):
                        nc.sync.dma_start(
                            wo_res[:, ct * c.DIM:(ct + 1) * c.DIM],
                            woT[:, ct * c.DIM:(ct + 1) * c.DIM])

                    with tc.tile_pool(name="attn", bufs=1) as attn, \
                         tc.tile_pool(name="spool", bufs=2, space="PSUM") as spool, \
                         tc.tile_pool(name="avp", bufs=4, space="PSUM") as avp, \
                         tc.tile_pool(name="ep", bufs=8) as ep, \
                         tc.tile_pool(name="np_", bufs=6) as np_:

                        def load_quarter(q):
                            """Collective-gated loads go through SWDGE
                            (Pool queue): every other queue has later work
                            that a waiting load would head-of-line block —
                            Pool is idle once the gathers are issued."""
                            nc.gpsimd.dma_start(
                                k_sb[q][:].rearrange("p (j t) -> p j t", j=4),
                                allg[q][:, 0:c.KQTR].rearrange(
                                    "j (p t) -> p j t", p=128))
                            for jj in range(4):
                                nc.gpsimd.dma_start(
                                    v65_sb[q][:, jj * 4 * 2 * V65:
                                              (jj + 1) * 4 * 2 * V65].rearrange(
                                        "p (t f) -> p t f", t=4),
                                    allg[q][jj, c.KQTR:c.QTR].rearrange(
                                        "(t p f) -> p t f", t=4, p=128))

                        for kv in range(c.NKV):
                            if kv % 2 == 0:
                                load_quarter(kv // 2)
                            f, r0 = kv // 2, (kv % 2) * 64
                            avs = [avp.tile([V65, c.TOK], F32, tag="av",
                                            name=f"av_{_rep}_{kv}_{i}")
                                   for i in range(c.GQ)]
                            for g in range(c.NKT):
                                jj, colbase = ktile_src(c, g)
                                m = jj * 4 + colbase // c.KT
                                kt = k_sb[f][r0:r0 + 64,
                                             jj * c.TOK + colbase:
                                             jj * c.TOK + colbase + c.KT]
                                v65t = v65_sb[f][:, m * 2 * V65 + (kv % 2) * V65:
                                                 m * 2 * V65 + (kv % 2) * V65 + V65]
                                a_live = g < c.NKT // 2
                                mk = mask_sb[:, None,
                                             g * c.BLK:(g + 1) * c.BLK
                                             ].broadcast_to([c.KT, 2, c.BLK])
                                if a_live:
                                    for p in range(2):
                                        sps = spool.tile([c.KT, 2 * c.TOK],
                                                         F32, tag="s")
                                        ex = ep.tile([c.KT, 2 * c.TOK],
                                                     BF16, tag="ex")
                                        exv = ex[:].rearrange(
                                            "k (i q) -> k i q", i=2)
                                        for hh in range(2):
                                            nc.tensor.matmul(
                                                sps[:, hh * c.TOK:hh * c.TOK + c.TOK],
                                                kt, q_ap(kv, 2 * p + hh),
                                                start=True, stop=True)
                                        nc.scalar.activation(
                                            ex[:], sps[:], AF.Exp, bias=0.0,
                                            scale=float(1.0 / np.sqrt(c.HD)))
                                        nc.vector.tensor_mul(
                                            exv[:, :, 0:c.BLK],
                                            exv[:, :, 0:c.BLK], mk)
                                        for hh in range(2):
                                            nc.tensor.matmul(
                                                avs[2 * p + hh][:],
                                                v65t, ex[:, hh * c.TOK:(hh + 1) * c.TOK],
                                                start=(g == 0), stop=(g == c.NKT - 1),
                                                skip_group_check=True)
                                else:
                                    # all 4 heads' block-B columns packed in
                                    # ONE sps tile: one exp, one mask op.
                                    sps = spool.tile([c.KT, 2 * c.TOK], F32,
                                                     tag="s")
                                    ex = ep.tile([c.KT, 2 * c.TOK], BF16,
                                                 tag="ex")
                                    for hq in range(c.GQ):
                                        nc.tensor.matmul(
                                            sps[:, hq * c.BLK:(hq + 1) * c.BLK],
                                            kt,
                                            q_ap(kv, hq)[:, c.BLK:2 * c.BLK],
                                            start=True, stop=True)
                                    nc.scalar.activation(
                                        ex[:], sps[:], AF.Exp, bias=0.0,
                                        scale=float(1.0 / np.sqrt(c.HD)))
                                    exv4 = ex[:].rearrange(
                                        "k (i q) -> k i q", i=4)
                                    nc.vector.tensor_mul(
                                        exv4[:], exv4[:],
                                        mask_sb[:, None,
                                                g * c.BLK:(g + 1) * c.BLK
                                                ].broadcast_to(
                                                    [c.KT, 4, c.BLK]))
                                    for hq in range(c.GQ):
                                        nc.tensor.matmul(
                                            avs[hq][:, c.BLK:2 * c.BLK],
                                            v65t,
                                            ex[:, hq * c.BLK:(hq + 1) * c.BLK],
                                            start=(g == 0), stop=(g == c.NKT - 1),
                                            skip_group_check=True)
                            # normalization: row 64 of av = softmax
                            # denominators. Broadcast 1/l to 64 partitions
                            # with a replicating SBUF->SBUF DMA so the norm
                            # never touches PE (keeps the in-order PE queue
                            # flowing into the next kv head's scores).
                            for hq in range(c.GQ):
                                hid = kv * c.GQ + hq
                                rr = np_.tile([1, c.TOK], F32, tag="rr")
                                nc.vector.reciprocal(rr[:], avs[hq][c.HD:V65, :])
                                bc_sb = np_.tile([c.HD, c.TOK], F32, tag="bc")
                                nc.sync.dma_start(
                                    bc_sb[:],
                                    rr[:, None, :].broadcast_to([1, c.HD, c.TOK]))
                                nc.vector.tensor_mul(yhead_ap(hid),
                                                     avs[hq][0:c.HD, :],
                                                     bc_sb[:])

                    # ================= phase 3: out projection =============
                    with tc.tile_pool(name="ops", bufs=2, space="PSUM") as ops, \
                         tc.tile_pool(name="osb", bufs=2) as osb:
                        for tt in range(c.TOK // 128):
                            po = ops.tile([128, c.DIM], F32, tag="o")
                            for ct in range(c.NCT):
                                for oc in range(c.DIM // 512):
                                    nc.tensor.matmul(
                                        po[:, oc * 512:(oc + 1) * 512],
                                        yT_sb[:, ct * c.TOK + tt * 128:
                                              ct * c.TOK + (tt + 1) * 128],
                                        wo_res[:, ct * c.DIM + oc * 512:
                                               ct * c.DIM + (oc + 1) * 512],
                                        start=(ct == 0), stop=(ct == c.NCT - 1))
                            o_sb = osb.tile([128, c.DIM], BF16, tag="ot")
                            nc.scalar.copy(o_sb[:], po[:])
                            nc.sync.dma_start(
                                out[tt * 128:(tt + 1) * 128, :], o_sb[:])

    return nc


# ---------------------------------------------------------------------------
# host side
# ---------------------------------------------------------------------------


def _rope_perm(n_heads, hd):
    p = []
    for h in range(n_heads):
        p.extend(h * hd + np.arange(0, hd, 2))
        p.extend(h * hd + np.arange(1, hd, 2))
    return np.array(p)


def _q_perm(hd):
    """Wq row order: slab (i, hq) = [head 8i+hq | head 8i+4+hq], each head
    as [evens; odds] — pairs an even-kv head with an odd-kv head so rope
    needs no cross-partition moves."""
    p = []
    for i in range(4):
        for hq in range(4):
            for h in (8 * i + hq, 8 * i + 4 + hq):
                p.extend(h * hd + np.arange(0, hd, 2))
                p.extend(h * hd + np.arange(1, hd, 2))
    return np.array(p)


def _cos_sin(positions, hd, base):
    inv = 1.0 / base ** (np.arange(0, hd, 2, dtype=np.float64) / hd)
    fr = np.outer(inv, positions.astype(np.float64))
    return np.cos(fr).astype(np.float32), np.sin(fr).astype(np.float32)


def _pretile_colslab(wT, nslab):
    """wT [DIM, O] -> [128, nslab * NCT*128]: slab p holds wT[:, p*128:+128]
    laid out [r, ct*128 + o] = wT[ct*128 + r, p*128 + o]."""
    dim, o_tot = wT.shape
    nct = dim // 128
    v = wT.reshape(nct, 128, nslab, 128)          # [ct, r, p, o]
    return np.ascontiguousarray(
        v.transpose(1, 2, 0, 3).reshape(128, nslab * nct * 128))


def _pretile_rowslab(wT):
    """wT [DIM, O] -> [128, NCT * O]: [r, ct*O + o] = wT[ct*128 + r, o]."""
    dim, o_tot = wT.shape
    nct = dim // 128
    v = wT.reshape(nct, 128, o_tot)               # [ct, r, o]
    return np.ascontiguousarray(
        v.transpose(1, 0, 2).reshape(128, nct * o_tot))


def make_inputs(cfg: Cfg, x, Wq, Wk, Wv, Wo):
    c = cfg
    permq = _q_perm(c.HD)
    permk = _rope_perm(c.NKV, c.HD)
    wqT = _pretile_colslab(Wq[permq].T.astype(BF), c.NH // 2)
    wkT = _pretile_colslab(Wk[permk].T.astype(BF), 4)
    wvT = _pretile_rowslab(Wv.T.astype(BF))
    woT = _pretile_rowslab(Wo.T.astype(BF))

    in_maps = []
    kk = np.arange(c.KT)
    for core in range(c.NCORES):
        b, jA, jB = core_blocks(core)
        toksA = np.arange(jA * c.BLK, (jA + 1) * c.BLK)
        toksB = np.arange(jB * c.BLK, (jB + 1) * c.BLK)
        toks = np.concatenate([toksA, toksB])
        xTc = _pretile_rowslab(x[b, toks, :].T.astype(BF))
        cos, sin = _cos_sin(toks, c.HD, c.rope_base)
        csa = np.concatenate([cos, sin, cos, sin], axis=0).astype(BF)
        csb = np.concatenate([sin, cos, sin, cos], axis=0).astype(BF)
        # masks: per k-tile g, the (A if g<8 else B)-block mask (broadcast
        # across the head pair on device).
        m = np.empty((c.KT, c.NKT * c.BLK), dtype=BF)
        for g in range(c.NKT):
            kpos = g * c.KT + kk
            blk_toks = toksA if g < c.NKT // 2 else toksB
            m[:, g * c.BLK:(g + 1) * c.BLK] = (
                kpos[:, None] <= blk_toks[None, :]).astype(BF)
        in_maps.append({
            "xT": xTc, "wqT": wqT, "wkT": wkT, "wvT": wvT, "woT": woT,
            "csA": csa, "csB": csb, "masks": m,
        })
    return in_maps


def assemble(cfg: Cfg, results):
    c = cfg
    out = np.empty((c.B, c.T, c.DIM), np.float32)
    for core in range(c.NCORES):
        b, jA, jB = core_blocks(core)
        o = results[core]["out"].astype(np.float32)
        out[b, jA * c.BLK:(jA + 1) * c.BLK] = o[0:c.BLK]
        out[b, jB * c.BLK:(jB + 1) * c.BLK] = o[c.BLK:2 * c.BLK]
    return out


_CACHE = {}


def kernel(x, Wq, Wk, Wv, Wo):
    _install_birfix()
    import os
    from concourse.bass_utils import run_bass_kernel_spmd

    cfg = FULL
    if "nc" not in _CACHE:
        _CACHE["nc"] = build_nc(cfg)
    nc = _CACHE["nc"]
    in_maps = make_inputs(cfg, np.asarray(x), np.asarray(Wq), np.asarray(Wk),
                          np.asarray(Wv), np.asarray(Wo))
    try:
        res = run_bass_kernel_spmd(nc, in_maps,
                                   core_ids=list(range(cfg.NCORES)))
    except ModuleNotFoundError:
        os.environ["BASS_NEVER_TRACE"] = "1"
        res = run_bass_kernel_spmd(nc, in_maps,
                                   core_ids=list(range(cfg.NCORES)))
    return assemble(cfg, res.results)



# revision 2
# speedup vs baseline: 1.1191x; 1.1191x over previous
"""Causal GQA attention block (RoPE, 32 q-heads / 8 kv-heads, fp32 I/O) on
8 Trainium2 NeuronCores — v3, head-parallel.

Sharding: core c owns batch b=c//4 and kv heads {2*(c%4), 2*(c%4)+1}
(= q heads 8*(c%4)..8*(c%4)+7) over the FULL 2048-token sequence. k/v
for the core's heads are computed locally from the full x — no k/v
all-gather. Causality is exploited uniformly across cores: scores/AV
for query block qb (512 tokens) only touch k-tiles 0..4*qb+3, cutting
attention matmul columns ~25% vs the dense superset with no per-core
program divergence.

The only collective is the out-projection reduction: each core's
partial out (contraction over its 8 heads' 512 y-dims) is
ReduceScattered (add) over the 4-core batch group in two 1024-token
chunks so the first RS overlaps the second half of the out-proj. Host
assemble() reorders the rank-interleaved token slices.

v ships with a ones-column per head (65-wide blocks) so the AV matmul
accumulates the softmax denominator for free (row 64).
"""

import sys
import json

sys.path.insert(0, "/opt/trn_rl_repo")

import numpy as np
import ml_dtypes

import concourse.bass as bass
import concourse.tile as tile
from concourse import mybir

F32 = mybir.dt.float32
BF16 = mybir.dt.bfloat16
BF = ml_dtypes.bfloat16
AF = mybir.ActivationFunctionType

# ---------------------------------------------------------------------------
# walrus workaround: this build supports one semaphore wait per instruction,
# but TileContext's tail drain attaches several. Split the extras onto
# standalone EventSemaphore instructions placed just before the instruction.
# ---------------------------------------------------------------------------


def _fix_multiwait(bir_bytes):
    d = json.loads(bir_bytes)
    ctr = 0
    changed = False
    for fn in d.get("functions", []):
        for blk in fn.get("blocks", []):
            new_insts = []
            for inst in blk["instructions"]:
                si = inst.get("sync_info") or {}
                waits = si.get("on_wait") or []
                if len(waits) > 1:
                    changed = True
                    for w in waits[:-1]:
                        ctr += 1
                        new_insts.append({
                            "debug": inst.get("debug", 0),
                            "engine": inst["engine"],
                            "ins": [],
                            "name": f"mwfix_{ctr}_{inst['name']}",
                            "opcode": "EventSemaphore",
                            "outs": [],
                            "sync_info": {"on_update": [], "on_wait": [w]},
                        })
                    si["on_wait"] = [waits[-1]]
                new_insts.append(inst)
            blk["instructions"] = new_insts
    return json.dumps(d).encode() if changed else bir_bytes


def _install_birfix():
    from concourse import bass_utils, bass2jax

    if getattr(bass_utils, "_mwfix_installed", False):
        return
    orig = bass_utils.compile_bir_kernel

    def patched(bir_json, tmpdir, neff_name="file.neff", **kw):
        if isinstance(bir_json, str):
            bir_json = bir_json.encode()
        return orig(_fix_multiwait(bir_json), tmpdir, neff_name, **kw)

    bass_utils.compile_bir_kernel = patched
    bass_utils._mwfix_installed = True
    bass2jax.compile_bir_kernel = patched


# ---------------------------------------------------------------------------
# configuration
# ---------------------------------------------------------------------------


class Cfg:
    def __init__(self, B=2, T=2048, DIM=2048, NH=32, NKV=8, HD=64,
                 rope_base=10000.0):
        self.B, self.T, self.DIM = B, T, DIM
        self.NH, self.NKV, self.HD = NH, NKV, HD
        self.rope_base = rope_base
        self.NCORES = 8
        self.NCT = DIM // 128         # contraction chunks (16)
        self.HD2 = HD // 2            # 32
        self.QB = 512                 # query block for causal skipping
        self.NQB = T // self.QB       # 4
        self.KT = 128                 # k-tile
        self.NKT = T // self.KT       # 16
        self.RSC = 2                  # reduce-scatter chunks
        self.RTOK = T // (4 * self.RSC)  # tokens per core per RS chunk (256)


FULL = Cfg()
V65 = FULL.HD + 1


# ---------------------------------------------------------------------------
# device program
# ---------------------------------------------------------------------------


def build_nc(cfg: Cfg, reps=1):
    c = cfg
    T = c.T
    nc = bass.Bass(num_devices=c.NCORES)

    # host-pre-tiled parameters (per-core data, shared program):
    #   xT    [128, ct*T + t]        = x[b]^T[ct*128+r, t]
    #   wqT   [128, p*2048 + ct*128+o]  4 head-pair col-slabs (rope-perm)
    #   wkT   [128, ct*128+o]        2 kv heads (rope-perm rows)
    #   wvT   [128, ct*128+o]        2 kv heads (natural rows)
    #   woT   [128, cc*2048 + o]     contraction rows = core's 512 y dims
    xT = nc.declare_dram_parameter("xT", [128, c.NCT * T], BF16, isOutput=False)
    wqT = nc.declare_dram_parameter("wqT", [128, 4 * c.NCT * 128], BF16,
                                    isOutput=False)
    wkT = nc.declare_dram_parameter("wkT", [128, c.NCT * 128], BF16,
                                    isOutput=False)
    wvT = nc.declare_dram_parameter("wvT", [128, c.NCT * 128], BF16,
                                    isOutput=False)
    woT = nc.declare_dram_parameter("woT", [128, 4 * c.DIM], BF16,
                                    isOutput=False)
    # rope tables (full seq): csA rows [cos,sin,cos,sin]x32, csB swapped.
    csA = nc.declare_dram_parameter("csA", [128, T], BF16, isOutput=False)
    csB = nc.declare_dram_parameter("csB", [128, T], BF16, isOutput=False)
    # masks[kk, g*512 + qq] = (g*128+kk <= (g//4)*512 + qq): the diagonal
    # block mask for k-tile g against its query block.
    masks = nc.declare_dram_parameter("masks", [c.KT, c.NKT * c.QB], BF16,
                                      isOutput=False)
    # out rows s*RTOK+i = sum over group of partial[s*(T//RSC) + rank*RTOK+i]
    out = nc.declare_dram_parameter("out", [T // 4, c.DIM], BF16,
                                    isOutput=True)

    TCH = T // c.RSC   # tokens per RS chunk (1024)
    partial = [nc.dram_tensor(f"partial{s}", [TCH, c.DIM], BF16)
               for s in range(c.RSC)]
    # collectives cannot write IO tensors: RS lands in internal scratch,
    # then a d2d DMA moves it to the external output.
    rs_out = [nc.dram_tensor(f"rsout{s}", [TCH // 4, c.DIM], BF16)
              for s in range(c.RSC)]

    groups = [[0, 1, 2, 3], [4, 5, 6, 7]]

    with tile.TileContext(nc) as tc:
        with tc.tile_pool(name="glob", bufs=1) as glob:
            qT_sb = glob.tile([128, 4 * T], BF16)          # pair p at p*T
            yT_sb = glob.tile([128, 4 * T], BF16)          # chunk cc at cc*T
            k_sb = glob.tile([128, T], BF16)               # kv at rows kv*64
            v65_sb = glob.tile([128, c.NKT * 2 * V65], BF16)
            csA_sb = glob.tile([128, T], BF16)
            csB_sb = glob.tile([128, T], BF16)
            mask_sb = glob.tile([c.KT, c.NKT * c.QB], BF16)
            nc.sync.dma_start(csA_sb[:], csA[:])
            nc.sync.dma_start(csB_sb[:], csB[:])
            nc.sync.dma_start(mask_sb[:], masks[:])

            def q_ap(kv, j, cols):
                """q head j (0..3) of kv head kv: [64, cols] at rows kv*64."""
                r = kv * 64
                return qT_sb[r:r + 64, j * T + cols[0]:j * T + cols[1]]

            def yhead_ap(L, cols):
                """y for local head L: rows (L%2)*64, col block (L//2)*T."""
                r = (L % 2) * 64
                return yT_sb[r:r + c.HD,
                             (L // 2) * T + cols[0]:(L // 2) * T + cols[1]]

            for _rep in range(reps):
                # ================= phase 1: projections + rope =============
                with tc.tile_pool(name="proj", bufs=1) as proj, \
                     tc.tile_pool(name="wstr", bufs=2) as wstr, \
                     tc.tile_pool(name="ppool", bufs=3, space="PSUM") as ppool, \
                     tc.tile_pool(name="vps", bufs=4, space="PSUM") as vps, \
                     tc.tile_pool(name="dr", bufs=6) as dr, \
                     tc.tile_pool(name="rt", bufs=2) as rt:

                    xT_sb = proj.tile([128, c.NCT * T], BF16)
                    wk_sb = proj.tile([128, c.NCT * 128], BF16)
                    wv_sb = proj.tile([128, c.NCT * 128], BF16)

                    nc.vector.memset(v65_sb[:], 1.0)
                    # wk piece 0 + x slab 0 first so the k-proj accumulation
                    # starts as soon as the first slabs land.
                    WSL = 4 * 128
                    nc.sync.dma_start(wk_sb[:, 0:WSL], wkT[:, 0:WSL])
                    SL = 2 * T
                    # first two x slabs are single-ct so the k-proj chain
                    # starts early; the rest ship as 2-ct slabs
                    nc.sync.dma_start(xT_sb[:, 0:T], xT[:, 0:T])
                    nc.sync.dma_start(xT_sb[:, T:SL], xT[:, T:SL])
                    for piece in range(1, 4):
                        nc.sync.dma_start(
                            wk_sb[:, piece * WSL:(piece + 1) * WSL],
                            wkT[:, piece * WSL:(piece + 1) * WSL])
                    for piece in range(1, 8):
                        nc.sync.dma_start(
                            xT_sb[:, piece * SL:(piece + 1) * SL],
                            xT[:, piece * SL:(piece + 1) * SL])
                    nc.sync.dma_start(wv_sb[:], wvT[:])

                    def xt_chunk(ct):
                        return xT_sb[:, ct * T:(ct + 1) * T]

                    TH = 512   # psum bank limit: matmul out <= 512 fp32 cols

                    # rope slices use absolute token columns th*512..+512
                    def rope_pair_th(ps, th, dst_of_hh):
                        n = TH
                        d = dr.tile([128, n], BF16, tag="d")
                        nc.scalar.copy(d[:], ps[:])
                        for hh in range(2):
                            ev = d[hh * 64:hh * 64 + 32, :]
                            od = d[hh * 64 + 32:hh * 64 + 64, :]
                            s0, s1 = th * n, (th + 1) * n
                            cos_e = csA_sb[hh * 64:hh * 64 + 32, s0:s1]
                            sin_o = csA_sb[hh * 64 + 32:hh * 64 + 64, s0:s1]
                            sin_e = csB_sb[hh * 64:hh * 64 + 32, s0:s1]
                            cos_o = csB_sb[hh * 64 + 32:hh * 64 + 64, s0:s1]
                            t1 = rt.tile([c.HD2, n], BF16, tag="t1")
                            t2 = rt.tile([c.HD2, n], BF16, tag="t2")
                            nc.vector.tensor_mul(t1[:], ev, cos_e)
                            nc.vector.tensor_mul(t2[:], od, sin_o)
                            nc.vector.tensor_sub(dst_of_hh(hh, 0), t1[:], t2[:])
                            t3 = rt.tile([c.HD2, n], BF16, tag="t1", name="t3")
                            t4 = rt.tile([c.HD2, n], BF16, tag="t2", name="t4")
                            nc.vector.tensor_mul(t3[:], ev, sin_e)
                            nc.vector.tensor_mul(t4[:], od, cos_o)
                            nc.vector.tensor_add(dst_of_hh(hh, 32), t3[:], t4[:])

                    # ---- k projection + rope (2 kv heads in one 128-row
                    # block), token quarters to fit one psum bank ----
                    for th in range(4):
                        ps = ppool.tile([128, TH], F32, tag="p")
                        for ct in range(c.NCT):
                            nc.tensor.matmul(
                                ps[:], wk_sb[:, ct * 128:(ct + 1) * 128],
                                xt_chunk(ct)[:, th * TH:(th + 1) * TH],
                                start=(ct == 0), stop=(ct == c.NCT - 1))

                        def kdst(hh, r0, th=th):
                            rr = hh * 64 + r0
                            return k_sb[rr:rr + 32, th * TH:(th + 1) * TH]
                        rope_pair_th(ps, th, kdst)

                    # ---- v projection (token-major out, ones col baked) ----
                    for tt in range(c.NKT):
                        psv = vps.tile([128, 128], F32, tag="v")
                        for ct in range(c.NCT):
                            nc.tensor.matmul(
                                psv[:],
                                xt_chunk(ct)[:, tt * 128:(tt + 1) * 128],
                                wv_sb[:, ct * 128:(ct + 1) * 128],
                                start=(ct == 0), stop=(ct == c.NCT - 1))
                        dst = v65_sb[:, tt * 2 * V65:(tt + 1) * 2 * V65]
                        nc.vector.tensor_copy(
                            dst.rearrange("p (i f) -> p i f", i=2)[:, :, 0:c.HD],
                            psv[:].rearrange("p (i f) -> p i f", i=2))

                    # ---- q projection + rope, head-pair slabs streamed ----
                    for p in range(4):
                        wq_sb = wstr.tile([128, c.NCT * 128], BF16, tag="wq")
                        nc.sync.dma_start(
                            wq_sb[:],
                            wqT[:, p * c.NCT * 128:(p + 1) * c.NCT * 128])
                        for th in range(4):
                            ps = ppool.tile([128, TH], F32, tag="p")
                            for ct in range(c.NCT):
                                nc.tensor.matmul(
                                    ps[:], wq_sb[:, ct * 128:(ct + 1) * 128],
                                    xt_chunk(ct)[:, th * TH:(th + 1) * TH],
                                    start=(ct == 0), stop=(ct == c.NCT - 1))

                            def qdst(hh, r0, p=p, th=th):
                                rr = hh * 64 + r0
                                col = p * T + th * TH
                                return qT_sb[rr:rr + 32, col:col + TH]
                            rope_pair_th(ps, th, qdst)

                # ================= phase 2: attention ======================
                with tc.tile_pool(name="wo", bufs=1) as wo_pool:
                    wo_res = wo_pool.tile([128, 4 * c.DIM], BF16)
                    for cc in range(4):
                        nc.sync.dma_start(
                            wo_res[:, cc * c.DIM:(cc + 1) * c.DIM],
                            woT[:, cc * c.DIM:(cc + 1) * c.DIM])

                    with tc.tile_pool(name="spool", bufs=2, space="PSUM") as spool, \
                         tc.tile_pool(name="avp", bufs=4, space="PSUM") as avp, \
                         tc.tile_pool(name="ep", bufs=6) as ep, \
                         tc.tile_pool(name="avs_p", bufs=8) as avs_p, \
                         tc.tile_pool(name="np_", bufs=6) as np_:

                        for kv in range(2):
                            r0 = kv * 64
                            for qb in range(c.NQB):
                                ng = 4 * qb + 4
                                q0, q1 = qb * c.QB, (qb + 1) * c.QB
                                avs = [avp.tile([V65, c.QB], F32, tag="av",
                                                name=f"av_{_rep}_{kv}_{qb}_{j}")
                                       for j in range(4)]
                                for g in range(ng):
                                    kt = k_sb[r0:r0 + 64,
                                              g * c.KT:(g + 1) * c.KT]
                                    v65t = v65_sb[:, (2 * g + kv) * V65:
                                                  (2 * g + kv + 1) * V65]
                                    diag = g >= 4 * qb
                                    for pp in range(2):
                                        sps = spool.tile([c.KT, 2 * c.QB],
                                                         F32, tag="s")
                                        for jj in range(2):
                                            nc.tensor.matmul(
                                                sps[:, jj * c.QB:(jj + 1) * c.QB],
                                                kt, q_ap(kv, 2 * pp + jj, (q0, q1)),
                                                start=True, stop=True)
                                        ex = ep.tile([c.KT, 2 * c.QB], BF16,
                                                     tag="ex")
                                        nc.scalar.activation(
                                            ex[:], sps[:], AF.Exp, bias=0.0,
                                            scale=float(1.0 / np.sqrt(c.HD)))
                                        if diag:
                                            exv = ex[:].rearrange(
                                                "k (i q) -> k i q", i=2)
                                            nc.vector.tensor_mul(
                                                exv[:], exv[:],
                                                mask_sb[:, None,
                                                        g * c.QB:(g + 1) * c.QB
                                                        ].broadcast_to(
                                                            [c.KT, 2, c.QB]))
                                        for jj in range(2):
                                            nc.tensor.matmul(
                                                avs[2 * pp + jj][:],
                                                v65t,
                                                ex[:, jj * c.QB:(jj + 1) * c.QB],
                                                start=(g == 0), stop=(g == ng - 1),
                                                skip_group_check=True)
                                # drain + normalize: row 64 = denominator.
                                # Drain on DVE so Act stays free for the
                                # next block's exps (PE waits on those).
                                for j in range(4):
                                    av_f = avs_p.tile([V65, c.QB], F32,
                                                      tag="avf")
                                    nc.vector.tensor_copy(av_f[:], avs[j][:])
                                    rr = np_.tile([1, c.QB], F32, tag="rr")
                                    nc.vector.reciprocal(
                                        rr[:], av_f[c.HD:V65, :])
                                    bc_sb = np_.tile([c.HD, c.QB], F32,
                                                     tag="bc")
                                    nc.sync.dma_start(
                                        bc_sb[:],
                                        rr[:, None, :].broadcast_to(
                                            [1, c.HD, c.QB]))
                                    nc.vector.tensor_mul(
                                        yhead_ap(4 * kv + j, (q0, q1)),
                                        av_f[0:c.HD, :], bc_sb[:])

                    # ================= phase 3: out proj + RS ==============
                    with tc.tile_pool(name="ops", bufs=2, space="PSUM") as ops, \
                         tc.tile_pool(name="osb", bufs=2) as osb:
                        for s in range(c.RSC):
                            for tl in range(T // c.RSC // 128):
                                tt = s * (T // c.RSC // 128) + tl
                                po = ops.tile([128, c.DIM], F32, tag="o")
                                for cc in range(4):
                                    for oc in range(c.DIM // 512):
                                        nc.tensor.matmul(
                                            po[:, oc * 512:(oc + 1) * 512],
                                            yT_sb[:, cc * T + tt * 128:
                                                  cc * T + (tt + 1) * 128],
                                            wo_res[:, cc * c.DIM + oc * 512:
                                                   cc * c.DIM + (oc + 1) * 512],
                                            start=(cc == 0), stop=(cc == 3))
                                o_sb = osb.tile([128, c.DIM], BF16, tag="ot")
                                nc.scalar.copy(o_sb[:], po[:])
                                nc.sync.dma_start(
                                    partial[s][tl * 128:(tl + 1) * 128, :],
                                    o_sb[:])
                            nc.gpsimd.collective_compute(
                                "ReduceScatter", mybir.AluOpType.add,
                                replica_groups=groups,
                                ins=[partial[s][:]],
                                outs=[rs_out[s][:]])
                            # Pool queue: idle after the RS is issued, so a
                            # collective-gated copy can't head-of-line block
                            # the SP queue's chunk-1 partial stores.
                            nc.gpsimd.dma_start(
                                out[s * (TCH // 4):(s + 1) * (TCH // 4), :],
                                rs_out[s][:])

    return nc


# ---------------------------------------------------------------------------
# host side
# ---------------------------------------------------------------------------


def _evod(h, hd):
    return list(h * hd + np.arange(0, hd, 2)) + list(h * hd + np.arange(1, hd, 2))


def _cos_sin(positions, hd, base):
    inv = 1.0 / base ** (np.arange(0, hd, 2, dtype=np.float64) / hd)
    fr = np.outer(inv, positions.astype(np.float64))
    return np.cos(fr).astype(np.float32), np.sin(fr).astype(np.float32)


def _pretile_colslab(wT, nslab):
    """wT [DIM, O] -> [128, nslab * NCT*128]: slab p holds wT[:, p*128:+128]
    laid out [r, ct*128 + o] = wT[ct*128 + r, p*128 + o]."""
    dim, o_tot = wT.shape
    nct = dim // 128
    v = wT.reshape(nct, 128, nslab, 128)          # [ct, r, p, o]
    return np.ascontiguousarray(
        v.transpose(1, 2, 0, 3).reshape(128, nslab * nct * 128))


def _pretile_rowslab(wT):
    """wT [DIM, O] -> [128, NCT * O]: [r, ct*O + o] = wT[ct*128 + r, o]."""
    dim, o_tot = wT.shape
    nct = dim // 128
    v = wT.reshape(nct, 128, o_tot)               # [ct, r, o]
    return np.ascontiguousarray(
        v.transpose(1, 0, 2).reshape(128, nct * o_tot))


def make_inputs(cfg: Cfg, x, Wq, Wk, Wv, Wo):
    c = cfg
    T = c.T
    cos, sin = _cos_sin(np.arange(T), c.HD, c.rope_base)
    csa = np.concatenate([cos, sin, cos, sin], axis=0).astype(BF)
    csb = np.concatenate([sin, cos, sin, cos], axis=0).astype(BF)
    kk = np.arange(c.KT)
    m = np.empty((c.KT, c.NKT * c.QB), dtype=BF)
    for g in range(c.NKT):
        kpos = g * c.KT + kk
        qpos = (g // 4) * c.QB + np.arange(c.QB)
        m[:, g * c.QB:(g + 1) * c.QB] = (
            kpos[:, None] <= qpos[None, :]).astype(BF)

    in_maps = []
    for core in range(c.NCORES):
        b, h4 = core // 4, core % 4
        xTc = _pretile_rowslab(x[b].T.astype(BF))
        # q: pair p = local heads (p, p+4), each [evens; odds]
        qrows = []
        for p in range(4):
            for L in (p, p + 4):
                qrows += _evod(8 * h4 + L, c.HD)
        wq_t = _pretile_colslab(Wq[np.array(qrows)].T.astype(BF), 4)
        krows = _evod(2 * h4, c.HD) + _evod(2 * h4 + 1, c.HD)
        wk_t = _pretile_colslab(Wk[np.array(krows)].T.astype(BF), 1)
        wv_t = _pretile_colslab(
            Wv[128 * h4:128 * (h4 + 1)].T.astype(BF), 1)
        wo_t = _pretile_rowslab(
            np.ascontiguousarray(Wo.T[512 * h4:512 * (h4 + 1)]).astype(BF))
        in_maps.append({
            "xT": xTc, "wqT": wq_t, "wkT": wk_t, "wvT": wv_t, "woT": wo_t,
            "csA": csa, "csB": csb, "masks": m,
        })
    return in_maps


def assemble(cfg: Cfg, results):
    c = cfg
    T = c.T
    out = np.empty((c.B, T, c.DIM), np.float32)
    for core in range(c.NCORES):
        b, rank = core // 4, core % 4
        o = results[core]["out"].astype(np.float32)
        for s in range(c.RSC):
            out[b, s * (T // c.RSC) + rank * c.RTOK:
                s * (T // c.RSC) + (rank + 1) * c.RTOK] = \
                o[s * c.RTOK:(s + 1) * c.RTOK]
    return out


_CACHE = {}


def kernel(x, Wq, Wk, Wv, Wo):
    _install_birfix()
    import os
    from concourse.bass_utils import run_bass_kernel_spmd

    cfg = FULL
    if "nc" not in _CACHE:
        _CACHE["nc"] = build_nc(cfg)
    nc = _CACHE["nc"]
    in_maps = make_inputs(cfg, np.asarray(x), np.asarray(Wq), np.asarray(Wk),
                          np.asarray(Wv), np.asarray(Wo))
    try:
        res = run_bass_kernel_spmd(nc, in_maps,
                                   core_ids=list(range(cfg.NCORES)))
    except ModuleNotFoundError:
        os.environ["BASS_NEVER_TRACE"] = "1"
        res = run_bass_kernel_spmd(nc, in_maps,
                                   core_ids=list(range(cfg.NCORES)))
    return assemble(cfg, res.results)
